# revision 4
# baseline (speedup 1.0000x reference)
"""Inverse 2D Haar wavelet transform (single-level idwt2) on 8 Trainium2 cores.

Full inputs: approximation/detail_h/detail_v/detail_d, each [8, 32, 256, 256] f32.
Full output: [8, 32, 512, 512] f32.

Sharding: batch dim across the 8 cores (fully data-parallel, no collectives).

Per-core kernel layout:
  Flatten (C, H) -> 8192 input rows of 256 f32.  For each input row r the two
  output plane rows (2i, 2i+1) are contiguous 1024 f32 in a [8192, 1024]
  "pair-row" view of the output, so stores are fully contiguous 4KB/partition.

  Butterfly per row block (DVE + ACT):
    s1 = A + H, d1 = A - H, s2 = V + D, d2 = V - D          (4x DVE tensor_tensor)
    s2h = 0.5*s2, d2h = 0.5*d2                              (2x ACT copy-with-scale)
    x00 = 0.5*s1 + s2h   -> out[..., 0, :, 0]               (4x DVE scalar_tensor_tensor,
    x01 = 0.5*s1 - s2h   -> out[..., 0, :, 1]                strided interleave writes)
    x10 = 0.5*d1 + d2h   -> out[..., 1, :, 0]
    x11 = 0.5*d1 - d2h   -> out[..., 1, :, 1]
"""

import sys

sys.path.insert(0, "/opt/trn_rl_repo")

import json

import numpy as np

import concourse.bass as bass
import concourse.mybir as mybir
from concourse.tile import TileContext
from concourse import bass_utils

F32 = mybir.dt.float32

B = 8          # batch (sharded across cores)
C = 32         # channels per core
H = 256        # coeff plane height
W = 256        # coeff plane width
ROWS = C * H   # 8192 flattened input rows per core
P = 128        # SBUF partitions
NBLK = 4       # 128-row blocks per super-tile (1 DMA load = 512KB, store = 2MB)
NSUP = ROWS // (P * NBLK)

_PATCHED = False

# Opcodes whose codegen struct has no room for inline sync waits in this
# walrus build (TPB_CTRL family).  All waits get hoisted off these.
_NO_INLINE_WAIT_OPCODES = {"Nop", "Drain"}


def _split_excess_waits(raw: bytes) -> bytes:
    """This container's walrus supports at most ONE inline sync wait per
    instruction ("Too many sync wait commands" otherwise), and none on
    Nop/Drain (except the eq-wait barrier Drains bass itself emits, which we
    leave untouched).  Hoist excess waits onto standalone EventSemaphore
    instructions inserted just before, on the same engine."""
    m = json.loads(raw)
    changed = False
    for fn in m["functions"]:
        for blk in fn["blocks"]:
            out = []
            for inst in blk["instructions"]:
                si = inst.get("sync_info")
                ow = (si or {}).get("on_wait") or []
                opc = inst.get("opcode", "")
                if opc in _NO_INLINE_WAIT_OPCODES:
                    # keep a single eq-imm wait (barrier pattern bass emits
                    # natively, which this walrus accepts); hoist the rest
                    keep = (
                        ow
                        if (
                            len(ow) == 1
                            and ow[0].get("wait_mode") == "sem-eq-imm"
                            and not (si.get("on_update") or [])
                        )
                        else []
                    )
                else:
                    keep = ow[-1:]
                if len(ow) > len(keep):
                    changed = True
                    for j, w in enumerate(ow[: len(ow) - len(keep)]):
                        out.append(
                            {
                                "debug": inst.get("debug"),
                                "engine": inst["engine"],
                                "ins": [],
                                "name": f"{inst['name']}-hoistw{j}",
                                "opcode": "EventSemaphore",
                                "outs": [],
                                "sync_info": {"on_update": [], "on_wait": [w]},
                            }
                        )
                    si["on_wait"] = ow[len(ow) - len(keep) :]
                out.append(inst)
            blk["instructions"] = out
    if not changed:
        return raw
    return json.dumps(m).encode()


def _patch_tile_tail():
    """This container's walrus rejects sync waits attached to Drain
    instructions ("Too many sync wait commands").  Re-emit the Tile tail as
    standalone EventSemaphore waits (1 wait per instruction) before a clean
    Drain; the butterfly barrier itself compiles fine (it is also emitted at
    kernel start by bass)."""
    global _PATCHED
    if _PATCHED:
        return
    _PATCHED = True

    def _drain_and_barrier(self, tick_clock, wait_clock):
        nc = self.nc
        gc = tick_clock.global_clock
        assert self.sems is not None
        for proc, sem in sorted(self.sems.allocated().items()):
            val = gc[proc]
            if val > 0:
                nc.sync.wait_ge(sem, val)
        nc.sync.drain()
        nc.all_engine_barrier()
        popped = nc._tile_sem_poison_stack.pop()
        assert popped is self._sem_poison
        nc.clear_and_free_semaphores(list(self.sems.allocated().values()))
        nc.all_engine_barrier()

    TileContext._drain_and_barrier = _drain_and_barrier

    orig_to_json_bytes = bass.Bass.to_json_bytes

    def to_json_bytes(self):
        return _split_excess_waits(orig_to_json_bytes(self))

    bass.Bass.to_json_bytes = to_json_bytes


def build_nc():
    _patch_tile_tail()
    nc = bass.Bass()
    a = nc.dram_tensor("a", [ROWS, W], F32, kind="ExternalInput")
    h = nc.dram_tensor("h", [ROWS, W], F32, kind="ExternalInput")
    v = nc.dram_tensor("v", [ROWS, W], F32, kind="ExternalInput")
    d = nc.dram_tensor("d", [ROWS, W], F32, kind="ExternalInput")
    o = nc.dram_tensor("o", [ROWS, 4 * W], F32, kind="ExternalOutput")

    # [P, ROWS/P, W]: partition p, block n  <->  flat row n*P + p
    av = a.ap().rearrange("(n p) w -> p n w", p=P)
    hv = h.ap().rearrange("(n p) w -> p n w", p=P)
    vv = v.ap().rearrange("(n p) w -> p n w", p=P)
    dv = d.ap().rearrange("(n p) w -> p n w", p=P)
    ov = o.ap().rearrange("(n p) w -> p n w", p=P)

    mult = mybir.AluOpType.mult
    add = mybir.AluOpType.add
    sub = mybir.AluOpType.subtract

    with TileContext(nc) as tc:
        with tc.tile_pool(name="io", bufs=3) as io_pool, tc.tile_pool(
            name="mid", bufs=2
        ) as mid_pool:
            for s in range(NSUP):
                n0 = s * NBLK
                ta = io_pool.tile([P, NBLK, W], F32, tag="ta")
                th = io_pool.tile([P, NBLK, W], F32, tag="th")
                tv = io_pool.tile([P, NBLK, W], F32, tag="tv")
                td = io_pool.tile([P, NBLK, W], F32, tag="td")
                nc.sync.dma_start(out=ta[:], in_=av[:, n0 : n0 + NBLK, :])
                nc.sync.dma_start(out=th[:], in_=hv[:, n0 : n0 + NBLK, :])
                nc.sync.dma_start(out=tv[:], in_=vv[:, n0 : n0 + NBLK, :])
                nc.sync.dma_start(out=td[:], in_=dv[:, n0 : n0 + NBLK, :])

                s1 = mid_pool.tile([P, NBLK, W], F32, tag="s1")
                d1 = mid_pool.tile([P, NBLK, W], F32, tag="d1")
                s2 = mid_pool.tile([P, NBLK, W], F32, tag="s2")
                d2 = mid_pool.tile([P, NBLK, W], F32, tag="d2")
                s2h = mid_pool.tile([P, NBLK, W], F32, tag="s2h")
                d2h = mid_pool.tile([P, NBLK, W], F32, tag="d2h")

                nc.vector.tensor_add(out=s1[:], in0=ta[:], in1=th[:])
                nc.vector.tensor_sub(out=d1[:], in0=ta[:], in1=th[:])
                nc.vector.tensor_add(out=s2[:], in0=tv[:], in1=td[:])
                nc.vector.tensor_sub(out=d2[:], in0=tv[:], in1=td[:])
                nc.scalar.mul(s2h[:], s2[:], 0.5)
                nc.scalar.mul(d2h[:], d2[:], 0.5)

                to = io_pool.tile([P, NBLK, 4 * W], F32, tag="to")
                # last dim 1024 = (row parity h, column w, x-pair parity t)
                tq = to.rearrange("p n (h w t) -> p n h w t", h=2, w=W, t=2)
                nc.vector.scalar_tensor_tensor(
                    out=tq[:, :, 0, :, 0], in0=s1[:], scalar=0.5, in1=s2h[:],
                    op0=mult, op1=add,
                )
                nc.vector.scalar_tensor_tensor(
                    out=tq[:, :, 0, :, 1], in0=s1[:], scalar=0.5, in1=s2h[:],
                    op0=mult, op1=sub,
                )
                nc.vector.scalar_tensor_tensor(
                    out=tq[:, :, 1, :, 0], in0=d1[:], scalar=0.5, in1=d2h[:],
                    op0=mult, op1=add,
                )
                nc.vector.scalar_tensor_tensor(
                    out=tq[:, :, 1, :, 1], in0=d1[:], scalar=0.5, in1=d2h[:],
                    op0=mult, op1=sub,
                )

                nc.sync.dma_start(out=ov[:, n0 : n0 + NBLK, :], in_=to[:])
    return nc


_NC_CACHE = None


def _get_nc():
    global _NC_CACHE
    if _NC_CACHE is None:
        _NC_CACHE = build_nc()
    return _NC_CACHE


def run_spmd(approximation, detail_h, detail_v, detail_d, **spmd_kwargs):
    ins = []
    for b in range(B):
        ins.append(
            {
                "a": np.ascontiguousarray(approximation[b], dtype=np.float32).reshape(ROWS, W),
                "h": np.ascontiguousarray(detail_h[b], dtype=np.float32).reshape(ROWS, W),
                "v": np.ascontiguousarray(detail_v[b], dtype=np.float32).reshape(ROWS, W),
                "d": np.ascontiguousarray(detail_d[b], dtype=np.float32).reshape(ROWS, W),
            }
        )
    res = bass_utils.run_bass_kernel_spmd(
        _get_nc(), ins, core_ids=list(range(B)), **spmd_kwargs
    )
    out = np.stack(
        [np.asarray(res.results[b]["o"]).reshape(C, 2 * H, 2 * W) for b in range(B)]
    )
    return out, res


def kernel(approximation, detail_h, detail_v, detail_d):
    out, _ = run_spmd(approximation, detail_h, detail_v, detail_d)
    return out


# revision 6
# speedup vs baseline: 1.2199x; 1.2199x over previous
"""Inverse 2D Haar wavelet transform (single-level idwt2) on 8 Trainium2 cores.

Full inputs: approximation/detail_h/detail_v/detail_d, each [8, 32, 256, 256] f32.
Full output: [8, 32, 512, 512] f32.

Sharding: batch dim across the 8 cores (fully data-parallel, no collectives).

Per-core kernel layout:
  Flatten (C, H) -> 8192 input rows of 256 f32.  For each input row r the two
  output plane rows (2i, 2i+1) are contiguous 1024 f32 in a [8192, 1024]
  "pair-row" view of the output, so stores are fully contiguous 4KB/partition.

  Butterfly per row block (DVE + ACT):
    s1 = A + H, d1 = A - H, s2 = V + D, d2 = V - D          (4x DVE tensor_tensor)
    s2h = 0.5*s2, d2h = 0.5*d2                              (2x ACT copy-with-scale)
    x00 = 0.5*s1 + s2h   -> out[..., 0, :, 0]               (4x DVE scalar_tensor_tensor,
    x01 = 0.5*s1 - s2h   -> out[..., 0, :, 1]                strided interleave writes)
    x10 = 0.5*d1 + d2h   -> out[..., 1, :, 0]
    x11 = 0.5*d1 - d2h   -> out[..., 1, :, 1]
"""

import sys

sys.path.insert(0, "/opt/trn_rl_repo")

import json

import numpy as np

import concourse.bass as bass
import concourse.mybir as mybir
from concourse.tile import TileContext
from concourse import bass_utils

F32 = mybir.dt.float32

B = 8          # batch (sharded across cores)
C = 32         # channels per core
H = 256        # coeff plane height
W = 256        # coeff plane width
ROWS = C * H   # 8192 flattened input rows per core
P = 128        # SBUF partitions
J = 4          # consecutive flat rows per partition (4KB load / 16KB store descriptors)
NSUP = ROWS // (P * J)  # 16 iterations, each: 512KB x4 loads, 2MB store

_PATCHED = False

# Opcodes whose codegen struct has no room for inline sync waits in this
# walrus build (TPB_CTRL family).  All waits get hoisted off these.
_NO_INLINE_WAIT_OPCODES = {"Nop", "Drain"}


def _split_excess_waits(raw: bytes) -> bytes:
    """This container's walrus supports at most ONE inline sync wait per
    instruction ("Too many sync wait commands" otherwise), and none on
    Nop/Drain (except the eq-wait barrier Drains bass itself emits, which we
    leave untouched).  Hoist excess waits onto standalone EventSemaphore
    instructions inserted just before, on the same engine."""
    m = json.loads(raw)
    changed = False
    for fn in m["functions"]:
        for blk in fn["blocks"]:
            out = []
            for inst in blk["instructions"]:
                si = inst.get("sync_info")
                ow = (si or {}).get("on_wait") or []
                opc = inst.get("opcode", "")
                if opc in _NO_INLINE_WAIT_OPCODES:
                    # keep a single eq-imm wait (barrier pattern bass emits
                    # natively, which this walrus accepts); hoist the rest
                    keep = (
                        ow
                        if (
                            len(ow) == 1
                            and ow[0].get("wait_mode") == "sem-eq-imm"
                            and not (si.get("on_update") or [])
                        )
                        else []
                    )
                else:
                    keep = ow[-1:]
                if len(ow) > len(keep):
                    changed = True
                    for j, w in enumerate(ow[: len(ow) - len(keep)]):
                        out.append(
                            {
                                "debug": inst.get("debug"),
                                "engine": inst["engine"],
                                "ins": [],
                                "name": f"{inst['name']}-hoistw{j}",
                                "opcode": "EventSemaphore",
                                "outs": [],
                                "sync_info": {"on_update": [], "on_wait": [w]},
                            }
                        )
                    si["on_wait"] = ow[len(ow) - len(keep) :]
                out.append(inst)
            blk["instructions"] = out
    if not changed:
        return raw
    return json.dumps(m).encode()


def _patch_tile_tail():
    """This container's walrus rejects sync waits attached to Drain
    instructions ("Too many sync wait commands").  Re-emit the Tile tail as
    standalone EventSemaphore waits (1 wait per instruction) before a clean
    Drain; the butterfly barrier itself compiles fine (it is also emitted at
    kernel start by bass)."""
    global _PATCHED
    if _PATCHED:
        return
    _PATCHED = True

    def _drain_and_barrier(self, tick_clock, wait_clock):
        nc = self.nc
        gc = tick_clock.global_clock
        assert self.sems is not None
        for proc, sem in sorted(self.sems.allocated().items()):
            val = gc[proc]
            if val > 0:
                nc.sync.wait_ge(sem, val)
        nc.sync.drain()
        nc.all_engine_barrier()
        popped = nc._tile_sem_poison_stack.pop()
        assert popped is self._sem_poison
        nc.clear_and_free_semaphores(list(self.sems.allocated().values()))
        nc.all_engine_barrier()

    TileContext._drain_and_barrier = _drain_and_barrier

    orig_to_json_bytes = bass.Bass.to_json_bytes

    def to_json_bytes(self):
        return _split_excess_waits(orig_to_json_bytes(self))

    bass.Bass.to_json_bytes = to_json_bytes


def build_nc():
    _patch_tile_tail()
    nc = bass.Bass()
    a = nc.dram_tensor("a", [ROWS, W], F32, kind="ExternalInput")
    h = nc.dram_tensor("h", [ROWS, W], F32, kind="ExternalInput")
    v = nc.dram_tensor("v", [ROWS, W], F32, kind="ExternalInput")
    d = nc.dram_tensor("d", [ROWS, W], F32, kind="ExternalInput")
    o = nc.dram_tensor("o", [ROWS, 4 * W], F32, kind="ExternalOutput")

    # Flat row r = i*(P*J) + p*J + j: iteration i, partition p, j one of J
    # consecutive rows.  Each partition's slice of a load is J*W*4 = 4KB
    # contiguous DRAM (16KB on the store side) for max DMA descriptor size.
    av = a.ap().rearrange("(i p j) w -> p i (j w)", p=P, j=J)
    hv = h.ap().rearrange("(i p j) w -> p i (j w)", p=P, j=J)
    vv = v.ap().rearrange("(i p j) w -> p i (j w)", p=P, j=J)
    dv = d.ap().rearrange("(i p j) w -> p i (j w)", p=P, j=J)
    ov = o.ap().rearrange("(i p j) w -> p i (j w)", p=P, j=J)

    mult = mybir.AluOpType.mult
    add = mybir.AluOpType.add
    sub = mybir.AluOpType.subtract
    FREE = J * W  # 1024 f32 per partition per input tile

    with TileContext(nc) as tc:
        with tc.tile_pool(name="io", bufs=3) as io_pool, tc.tile_pool(
            name="mid", bufs=2
        ) as mid_pool:
            for i in range(NSUP):
                ta = io_pool.tile([P, FREE], F32, tag="ta")
                th = io_pool.tile([P, FREE], F32, tag="th")
                tv = io_pool.tile([P, FREE], F32, tag="tv")
                td = io_pool.tile([P, FREE], F32, tag="td")
                nc.sync.dma_start(out=ta[:], in_=av[:, i, :])
                nc.sync.dma_start(out=th[:], in_=hv[:, i, :])
                nc.sync.dma_start(out=tv[:], in_=vv[:, i, :])
                nc.sync.dma_start(out=td[:], in_=dv[:, i, :])

                s1 = mid_pool.tile([P, FREE], F32, tag="s1")
                d1 = mid_pool.tile([P, FREE], F32, tag="d1")
                s2 = mid_pool.tile([P, FREE], F32, tag="s2")
                d2 = mid_pool.tile([P, FREE], F32, tag="d2")
                s2h = mid_pool.tile([P, FREE], F32, tag="s2h")
                d2h = mid_pool.tile([P, FREE], F32, tag="d2h")

                nc.vector.tensor_add(out=s1[:], in0=ta[:], in1=th[:])
                nc.vector.tensor_sub(out=d1[:], in0=ta[:], in1=th[:])
                nc.vector.tensor_add(out=s2[:], in0=tv[:], in1=td[:])
                nc.vector.tensor_sub(out=d2[:], in0=tv[:], in1=td[:])
                nc.scalar.mul(s2h[:], s2[:], 0.5)
                nc.scalar.mul(d2h[:], d2[:], 0.5)

                to = io_pool.tile([P, 4 * FREE], F32, tag="to")
                # output free layout: j * 1024 + h*512 + w*2 + t
                tq = to.rearrange("p (j h w t) -> p j h w t", j=J, h=2, w=W, t=2)
                s1v = s1.rearrange("p (j w) -> p j w", j=J)
                d1v = d1.rearrange("p (j w) -> p j w", j=J)
                s2v = s2h.rearrange("p (j w) -> p j w", j=J)
                d2v = d2h.rearrange("p (j w) -> p j w", j=J)
                nc.vector.scalar_tensor_tensor(
                    out=tq[:, :, 0, :, 0], in0=s1v[:], scalar=0.5, in1=s2v[:],
                    op0=mult, op1=add,
                )
                nc.vector.scalar_tensor_tensor(
                    out=tq[:, :, 0, :, 1], in0=s1v[:], scalar=0.5, in1=s2v[:],
                    op0=mult, op1=sub,
                )
                nc.vector.scalar_tensor_tensor(
                    out=tq[:, :, 1, :, 0], in0=d1v[:], scalar=0.5, in1=d2v[:],
                    op0=mult, op1=add,
                )
                nc.vector.scalar_tensor_tensor(
                    out=tq[:, :, 1, :, 1], in0=d1v[:], scalar=0.5, in1=d2v[:],
                    op0=mult, op1=sub,
                )

                # store on the ACT HWDGE ring so loads (SP ring) and stores
                # round-robin across both physical HW-DGE queues
                nc.scalar.dma_start(out=ov[:, i, :], in_=to[:])
    return nc


_NC_CACHE = None


def _get_nc():
    global _NC_CACHE
    if _NC_CACHE is None:
        _NC_CACHE = build_nc()
    return _NC_CACHE


def run_spmd(approximation, detail_h, detail_v, detail_d, **spmd_kwargs):
    ins = []
    for b in range(B):
        ins.append(
            {
                "a": np.ascontiguousarray(approximation[b], dtype=np.float32).reshape(ROWS, W),
                "h": np.ascontiguousarray(detail_h[b], dtype=np.float32).reshape(ROWS, W),
                "v": np.ascontiguousarray(detail_v[b], dtype=np.float32).reshape(ROWS, W),
                "d": np.ascontiguousarray(detail_d[b], dtype=np.float32).reshape(ROWS, W),
            }
        )
    res = bass_utils.run_bass_kernel_spmd(
        _get_nc(), ins, core_ids=list(range(B)), **spmd_kwargs
    )
    out = np.stack(
        [np.asarray(res.results[b]["o"]).reshape(C, 2 * H, 2 * W) for b in range(B)]
    )
    return out, res


def kernel(approximation, detail_h, detail_v, detail_d):
    out, _ = run_spmd(approximation, detail_h, detail_v, detail_d)
    return out
